# revision 1
# baseline (speedup 1.0000x reference)
"""GAT layer (nn_GATLayer_88579405512952) — Trainium2 Bass kernel, 8 NeuronCores.

Math (reference):
    Wh  = h @ W                      [N, D]
    Wh1 = Wh @ a[:D],  Wh2 = Wh @ a[D:]
    e[i,j] = leaky_relu(Wh1[i] + Wh2[j], 0.2)       (rank-1 + pointwise)
    out = elu(softmax_row(e) @ Wh)
    (adj is unused by the reference; we never touch it.)

Key algebraic transform used here:
    exp(leaky_relu(s)) = exp(max(s, 0.2 s)) = max(exp(s), exp(0.2 s))
    and softmax rows are invariant to any positive per-row scale, so with
      R1[i] = exp(0.8*Wh1[i]),  E2[j] = exp(Wh2[j]),  E2a[j] = exp(0.2*Wh2[j])
    the unnormalized attention  w'[i,j] = max(R1[i]*E2[j], E2a[j])
    gives exactly softmax(e) after row-normalization. This removes every
    transcendental from the N^2 inner loop: one fused 2-op DVE tensor_scalar
    per [128 x 1024] tile. The row-sum (softmax denominator) is obtained for
    free by augmenting Wh with a ones column inside the PE matmul.

The whole N^2 datapath runs in bf16 (verified 2.3e-3 rel err vs the fp32
reference on the real input distribution): bf16 doubles DVE throughput
(4x perf mode vs 2x for fp32 tensor_scalar) and halves the hT DMA bytes.
PSUM accumulation stays fp32 throughout.

Every ACT_PERIOD-th score tile is computed on the otherwise-idle scalar
engine via the exact decomposition max(a,b) = b + relu(a-b):
    w~ = relu(E2[j]*R1[i] - E2a[j]) = w - E2a[j]
one Relu activation with per-partition scale/bias.  The missing rank-1 term
sum_j E2a[j]*Whaug[j,:] is accumulated by tiny matmuls and added back during
the PSUM->SBUF epilogue copy as a per-partition bias — so the DVE and the
scalar engine chew the N^2 elementwise work in parallel.

Sharding: each core owns 1024 rows i (flash-attention style 1D row shard),
computes its [1024 x 8192] score block on-chip (never materialized in HBM),
and produces out[c*1024:(c+1)*1024, :]. Wh/E2 are computed redundantly per
core from hT (4 MB bf16) — cheaper and simpler than an all-gather.

The program is emitted batch-interleaved (Wh-projection batch b, then
attention batch b-LAG) so the PE never head-of-line blocks on the hT DMA:
projection matmuls for later chunks interleave with attention matmuls for
earlier ones.

Host-side marshalling (layout only; all FLOPs on device): h is passed
transposed (hT, bf16) so the PE can contract over the feature dim, and the
tiny [256,64]@[64,1] param products W@a1, W@a2 are folded into an augmented
weight matrix (constant folding of parameters).
"""

import functools
import os

import numpy as np

# Recovers wedged NeuronCores (NRT_EXEC_UNIT_UNRECOVERABLE) on process start;
# benign for healthy devices.  Must be set before the runtime initializes.
os.environ.setdefault("NEURON_RT_RESET_CORES", "1")

N = 8192
IN_DIM = 256
OUT_DIM = 64
ALPHA = 0.2
NCORES = 8
ROWS = N // NCORES          # 1024 rows per core
P = 128
JT = N // P                 # 64 j-tiles
KC = IN_DIM // P            # 2 contraction chunks
DA = OUT_DIM + 1            # 65 = [Wh | ones]
WAUGC = OUT_DIM + 4         # 68 = [W | wa1 | wa2 | pad pad]
BATCH = 7                   # j-tiles per Wh-projection batch (7*68 <= 512)
LAG = 2                     # attention batch lags projection by this much
NCH = 8                     # hT DMA chunks
ACT_PERIOD = 7              # tiles with t % ACT_PERIOD == ACT_PHASE go to the
ACT_PHASE = 3               # scalar engine (relu-decomposition offload)
WPOOL_BUFS = 6
EPOOL_BUFS = 2

_BSIZES = [BATCH] * (JT // BATCH) + ([JT % BATCH] if JT % BATCH else [])
_BSTARTS = [sum(_BSIZES[:i]) for i in range(len(_BSIZES))]
NB = len(_BSIZES)


def build_nc(repeat: int = 1):
    """Build the Bass program (same NEFF for all 8 cores).

    repeat > 1 re-issues the whole pipeline (DMA included) that many times —
    used by test.py for delta wall-clock timing of the hardware kernel.
    """
    import concourse.mybir as mybir
    import concourse.tile as tile
    from concourse import bacc
    from concourse.masks import make_identity

    fp32 = mybir.dt.float32
    bf16 = mybir.dt.bfloat16
    Alu = mybir.AluOpType
    Act = mybir.ActivationFunctionType

    nc = bacc.Bacc("TRN2", target_bir_lowering=False, debug=False,
                   num_devices=NCORES)

    hT_d = nc.dram_tensor("hT", [IN_DIM, N], bf16, kind="ExternalInput")
    hTo_d = nc.dram_tensor("hTo", [IN_DIM, ROWS], bf16, kind="ExternalInput")
    waug_d = nc.dram_tensor("waug", [IN_DIM, WAUGC], bf16,
                            kind="ExternalInput")
    out_d = nc.dram_tensor("out", [ROWS, OUT_DIM], fp32, kind="ExternalOutput")

    hT_r = hT_d.ap().rearrange("(c p) j -> p c j", p=P)        # [128, 2, 8192]
    hTo_r = hTo_d.ap().rearrange("(c p) i -> p c i", p=P)      # [128, 2, 1024]
    waug_r = waug_d.ap().rearrange("(c p) d -> p c d", p=P)    # [128, 2, 68]
    out_r = out_d.ap().rearrange("(b p) d -> p b d", p=P)      # [128, 8, 64]

    with tile.TileContext(nc) as tc:
        with (
            tc.tile_pool(name="singles", bufs=1) as singles,
            tc.tile_pool(name="vpool", bufs=1) as vpool,
            tc.tile_pool(name="hpool", bufs=1) as hpool,
            tc.tile_pool(name="wpool", bufs=WPOOL_BUFS) as wpool,
            tc.tile_pool(name="epool", bufs=EPOOL_BUFS) as epool,
            tc.tile_pool(name="ps_wh", bufs=2, space="PSUM") as ps_wh,
            tc.tile_pool(name="ps_acc", bufs=1, space="PSUM") as ps_acc,
            tc.tile_pool(name="ps_misc", bufs=1, space="PSUM") as ps_misc,
            tc.tile_pool(name="ps_tr", bufs=2, space="PSUM") as ps_tr,
        ):
            identity = singles.tile([P, P], fp32)
            make_identity(nc, identity)

            for _rep in range(repeat):
                # ---- load inputs --------------------------------------
                waug_sb = hpool.tile([P, KC, WAUGC], bf16, tag="waug")
                nc.sync.dma_start(waug_sb[:], waug_r)
                hTo_sb = hpool.tile([P, KC, ROWS], bf16, tag="hTo")
                nc.sync.dma_start(hTo_sb[:], hTo_r)
                hT_sb = hpool.tile([P, KC, N], bf16, tag="hT")
                CW = N // NCH
                for s in range(NCH):
                    nc.sync.dma_start(
                        hT_sb[:, :, s * CW:(s + 1) * CW],
                        hT_r[:, :, s * CW:(s + 1) * CW],
                    )

                # ---- R1_bcast[p, i] = exp(0.8 * Wh1[i]) for own rows ----
                # Wh1_bcast via matmul with the Wa1 column broadcast to all
                # 128 weight columns -> identical value in every partition.
                ps_bc = ps_misc.tile([P, ROWS], fp32, tag="misc")
                wa1_rep = wpool.tile([P, KC, P], bf16, tag="wa1rep")
                for c in range(KC):
                    nc.vector.tensor_copy(
                        wa1_rep[:, c, :],
                        waug_sb[:, c, OUT_DIM:OUT_DIM + 1].to_broadcast(
                            [P, P]))
                for c in range(KC):
                    for half in range(2):
                        sl = slice(half * 512, (half + 1) * 512)
                        nc.tensor.matmul(
                            ps_bc[:, sl], wa1_rep[:, c, :], hTo_sb[:, c, sl],
                            start=(c == 0), stop=(c == KC - 1),
                        )
                r1b = vpool.tile([P, ROWS], bf16, tag="r1b")
                nc.scalar.activation(r1b[:], ps_bc[:], Act.Exp, scale=0.8)

                # ---- interleaved: Wh projection + attention batches -----
                # v_all[:, t*65:(t+1)*65] = [Wh_t | ones] (bf16)
                v_all = vpool.tile([P, JT * DA], bf16, tag="v_all")
                v_r = v_all.rearrange("p (t d) -> p t d", d=DA)
                nc.vector.memset(v_r[:, :, OUT_DIM], 1.0)
                # scalar operands of tensor_scalar must be fp32
                e2 = vpool.tile([P, JT], fp32, tag="e2")
                e2a = vpool.tile([P, JT], fp32, tag="e2a")

                acc0 = ps_acc.tile([DA, 512], fp32, tag="acc0")
                acc1 = ps_acc.tile([DA, 512], fp32, tag="acc1")
                # rank-1 correction for scalar-engine tiles:
                # corr[d] = sum_{j in act tiles} E2a[j] * Whaug[j, d]
                # (reuses the r1-phase PSUM bank; start=True resets it)
                corr_full = ps_misc.tile([P, ROWS], fp32, tag="misc",
                                         name="corr")
                corr_ps = corr_full[0:DA, 0:1]
                act_tiles = [t for t in range(JT)
                             if t % ACT_PERIOD == ACT_PHASE]

                def wh_batch(b):
                    # projection for j-tiles of batch b: matmuls into one
                    # PSUM bank, one batched copy to SBUF, exps from PSUM.
                    t0, bs = _BSTARTS[b], _BSIZES[b]
                    ps = ps_wh.tile([P, BATCH, WAUGC], fp32, tag="wh")
                    for k in range(bs):
                        t = t0 + k
                        for c in range(KC):
                            nc.tensor.matmul(
                                ps[:, k, :],
                                hT_sb[:, c, t * P:(t + 1) * P],
                                waug_sb[:, c, :],
                                start=(c == 0), stop=(c == KC - 1),
                            )
                    ts = slice(t0, t0 + bs)
                    nc.scalar.activation(v_r[:, ts, 0:OUT_DIM],
                                         ps[:, 0:bs, 0:OUT_DIM], Act.Copy)
                    nc.scalar.activation(e2[:, ts],
                                         ps[:, 0:bs, OUT_DIM + 1], Act.Exp)
                    nc.scalar.activation(e2a[:, ts],
                                         ps[:, 0:bs, OUT_DIM + 1], Act.Exp,
                                         scale=ALPHA)

                def attn_batch(b):
                    # scores + matmul accumulation for j-tiles of batch b.
                    # Most tiles: fused mult+max on DVE.  Every ACT_PERIOD-th
                    # tile runs on the scalar engine instead, as
                    #   w~ = relu(E2[j]*R1[i] - E2a[j]) = w - E2a[j],
                    # whose missing rank-1 part is accumulated into corr_ps
                    # and added back in the epilogue as a per-partition bias.
                    t0, bs = _BSTARTS[b], _BSIZES[b]
                    for k in range(bs):
                        t = t0 + k
                        w = wpool.tile([P, ROWS], bf16, tag="w")
                        if t % ACT_PERIOD == ACT_PHASE:
                            nege2a = wpool.tile([P, 1], fp32, tag="nege2a")
                            nc.vector.tensor_scalar(
                                nege2a[:], e2a[:, t:t + 1], -1.0, None,
                                Alu.mult)
                            nc.scalar.activation(
                                w[:], r1b[:], Act.Relu,
                                bias=nege2a[:], scale=e2[:, t:t + 1])
                            e2a16 = wpool.tile([P, 1], bf16, tag="e2a16")
                            nc.vector.tensor_copy(e2a16[:], e2a[:, t:t + 1])
                            nc.tensor.matmul(
                                corr_ps, v_r[:, t, :], e2a16[:],
                                start=(t == act_tiles[0]),
                                stop=(t == act_tiles[-1]))
                        else:
                            nc.vector.tensor_scalar(
                                w[:], r1b[:],
                                e2[:, t:t + 1], e2a[:, t:t + 1],
                                Alu.mult, Alu.max,
                            )
                        nc.tensor.matmul(acc0[:], v_r[:, t, :], w[:, 0:512],
                                         start=(t == 0), stop=(t == JT - 1))
                        nc.tensor.matmul(acc1[:], v_r[:, t, :], w[:, 512:1024],
                                         start=(t == 0), stop=(t == JT - 1))

                for b in range(NB + LAG):
                    if b < NB:
                        wh_batch(b)
                    if b >= LAG:
                        attn_batch(b - LAG)

                # ---- epilogue: normalize, ELU, transpose, store ---------
                # the PSUM->SBUF copy adds the act-tile correction vector
                # for free via the per-partition bias of Identity.
                corr_sb = epool.tile([DA, 1], fp32, tag="corr_sb")
                nc.scalar.activation(corr_sb[:], corr_ps, Act.Identity)
                numt = epool.tile([DA, ROWS], fp32, tag="numt")
                nc.scalar.activation(numt[:, 0:512], acc0[:], Act.Identity,
                                     bias=corr_sb[:])
                nc.scalar.activation(numt[:, 512:1024], acc1[:], Act.Identity,
                                     bias=corr_sb[:])

                out_all = epool.tile([P, ROWS // P, OUT_DIM], fp32, tag="oall")
                for bb in range(ROWS // P):
                    ps_t = ps_tr.tile([P, DA], fp32, tag="tr", name="ps_t")
                    nc.tensor.transpose(ps_t[:], numt[:, bb * P:(bb + 1) * P],
                                        identity[0:DA, 0:DA])
                    zinv = wpool.tile([P, 1], fp32, tag="zinv")
                    nc.vector.reciprocal(zinv[:], ps_t[:, OUT_DIM:DA])
                    # division stays on the DVE: the scalar engine is the
                    # busier of the two here (measured — moving this to Act
                    # Identity+scale regressed, as did a larger act-tile share)
                    nc.vector.tensor_scalar(
                        out_all[:, bb, :], ps_t[:, 0:OUT_DIM], zinv[:], None,
                        Alu.mult,
                    )

                # ELU, exactly: (max(x,0) - 1) + exp(min(x,0))
                flat = out_all.rearrange("p b d -> p (b d)")
                r = epool.tile([P, ROWS // P * OUT_DIM], fp32, tag="elur")
                m = epool.tile([P, ROWS // P * OUT_DIM], fp32, tag="elum")
                nc.vector.tensor_scalar(r[:], flat, 0.0, -1.0, Alu.max, Alu.add)
                nc.vector.tensor_scalar(m[:], flat, 0.0, None, Alu.min)
                nc.scalar.activation(m[:], m[:], Act.Exp)
                nc.vector.tensor_tensor(flat, r[:], m[:], Alu.add)

                nc.sync.dma_start(out_r, out_all[:])

    nc.compile()
    return nc


@functools.lru_cache(maxsize=4)
def _cached_nc(repeat: int = 1):
    return build_nc(repeat)


class _Runner:
    """Compile once, load once, execute many times on the 8 cores.

    Mirrors concourse.bass2jax.run_bass_via_pjrt's multi-core path but caches
    the jitted executable and the device-resident inputs, so repeated calls
    measure (dispatch + device execution) only.  Output tensors are fully
    written by the kernel, so the zero "donation" buffers are passed as
    ordinary (cached) params without donation.
    """

    def __init__(self, repeat: int = 1):
        import jax
        from jax.experimental.shard_map import shard_map
        from jax.sharding import Mesh, NamedSharding, PartitionSpec
        import concourse.mybir as mybir
        from concourse import bass2jax

        self.jax = jax
        nc = _cached_nc(repeat)
        partition_name = (nc.partition_id_tensor.name
                          if nc.partition_id_tensor else None)
        bass2jax.install_neuronx_cc_hook()

        in_names, out_names, out_avals, zero_outs = [], [], [], []
        for alloc in nc.m.functions[0].allocations:
            if not isinstance(alloc, mybir.MemoryLocationSet):
                continue
            name = alloc.memorylocations[0].name
            if alloc.kind == "ExternalInput":
                if name != partition_name:
                    in_names.append(name)
            elif alloc.kind == "ExternalOutput":
                shape = tuple(alloc.tensor_shape)
                dt = mybir.dt.np(alloc.dtype)
                out_names.append(name)
                out_avals.append(jax.core.ShapedArray(shape, dt))
                zero_outs.append(np.zeros((NCORES * shape[0], *shape[1:]), dt))
        self.in_names = in_names
        self.out_names = out_names
        self.out_shapes = [tuple(a.shape) for a in out_avals]
        all_names = tuple(in_names + out_names)
        if partition_name is not None:
            all_names = all_names + (partition_name,)

        def _body(*args):
            operands = list(args)
            if partition_name is not None:
                operands.append(bass2jax.partition_id_tensor())
            outs = bass2jax._bass_exec_p.bind(
                *operands,
                out_avals=tuple(out_avals),
                in_names=all_names,
                out_names=tuple(out_names),
                lowering_input_output_aliases=(),
                sim_require_finite=True,
                sim_require_nnan=True,
                nc=nc,
            )
            return tuple(outs)

        devices = jax.devices()[:NCORES]
        mesh = Mesh(np.asarray(devices), ("core",))
        n_args = len(in_names) + len(out_names)
        self.fn = jax.jit(
            shard_map(
                _body, mesh=mesh,
                in_specs=(PartitionSpec("core"),) * n_args,
                out_specs=(PartitionSpec("core"),) * len(out_names),
                check_rep=False,
            ),
            keep_unused=True,
        )
        self.sharding = NamedSharding(mesh, PartitionSpec("core"))
        self.zero_dev = [jax.device_put(z, self.sharding) for z in zero_outs]
        self.dev_inputs = None
        self._inputs_key = None

    def set_inputs(self, in_maps):
        key = id(in_maps)
        if self._inputs_key == key and self.dev_inputs is not None:
            return
        concat = [
            np.concatenate([np.asarray(m[name]) for m in in_maps], axis=0)
            for name in self.in_names
        ]
        self.dev_inputs = [
            self.jax.device_put(c, self.sharding) for c in concat
        ]
        self.jax.block_until_ready(self.dev_inputs)
        self._inputs_key = key

    def execute(self):
        outs = self.fn(*self.dev_inputs, *self.zero_dev)
        self.jax.block_until_ready(outs)
        return outs

    def results(self):
        outs = self.execute()
        per_core = []
        for c in range(NCORES):
            per_core.append({
                name: np.asarray(outs[i]).reshape(
                    NCORES, *self.out_shapes[i])[c]
                for i, name in enumerate(self.out_names)
            })
        return per_core


@functools.lru_cache(maxsize=4)
def _cached_runner(repeat: int = 1):
    return _Runner(repeat)


def _bf16_np():
    import concourse.mybir as mybir
    return mybir.dt.np(mybir.dt.bfloat16)


def _marshal(h, W, a):
    bf = _bf16_np()
    h = np.asarray(h, dtype=np.float32)
    W = np.asarray(W, dtype=np.float32)
    a = np.asarray(a, dtype=np.float32).reshape(2 * OUT_DIM, 1)
    hT = np.ascontiguousarray(h.T).astype(bf)          # [256, 8192] bf16
    wa1 = W @ a[:OUT_DIM]                              # [256, 1]
    wa2 = W @ a[OUT_DIM:]                              # [256, 1]
    waug = np.ascontiguousarray(
        np.concatenate([W, wa1, wa2, np.zeros((IN_DIM, 2), np.float32)],
                       axis=1)).astype(bf)             # [256, 68] bf16
    in_maps = []
    for c in range(NCORES):
        in_maps.append({
            "hT": hT,
            "hTo": np.ascontiguousarray(hT[:, c * ROWS:(c + 1) * ROWS]),
            "waug": waug,
        })
    return in_maps


def run_on_cores(in_maps, repeat: int = 1):
    runner = _cached_runner(repeat)
    runner.set_inputs(in_maps)
    return runner.results()


def _run_fallback(in_maps):
    """Slow-but-blessed execution path (fresh compile each call)."""
    from concourse.bass_utils import run_bass_kernel_spmd
    nc = build_nc(1)
    res = run_bass_kernel_spmd(nc, in_maps, core_ids=list(range(NCORES)))
    return res.results


def kernel(h, adj, W, a):
    import time
    in_maps = _marshal(h, W, a)
    res = None
    last_exc = None
    for attempt in range(4):
        try:
            if attempt < 3:
                res = run_on_cores(in_maps, repeat=1)
            else:
                res = _run_fallback(in_maps)
            break
        except Exception as e:  # device wedge etc: wait for recovery, retry
            last_exc = e
            _cached_runner.cache_clear()
            _cached_nc.cache_clear()
            time.sleep(20 * (attempt + 1))
    if res is None:
        raise last_exc
    out = np.concatenate([r["out"] for r in res], axis=0)
    return out.astype(np.float32)


if __name__ == "__main__":
    rng = np.random.default_rng(0)
    h = rng.standard_normal((N, IN_DIM), dtype=np.float32)
    W = (rng.standard_normal((IN_DIM, OUT_DIM), dtype=np.float32) * 0.1)
    a = (rng.standard_normal((2 * OUT_DIM, 1), dtype=np.float32) * 0.1)
    adj = np.zeros((N, N), dtype=bool)
    out = kernel(h, adj, W, a)
    print("out", out.shape, out.dtype, float(out.mean()))



# revision 4
# speedup vs baseline: 2.2652x; 2.2652x over previous
"""GAT layer (nn_GATLayer_88579405512952) — Trainium2 Bass kernel, 8 NeuronCores.

Sort-prefix algorithm (replaces the O(N^2) dense attention datapath):

    e[i,j] = leaky_relu(Wh1[i] + Wh2[j]);  softmax rows; @ Wh.
    With exp(leaky_relu(s)) = max(e^s, e^{0.2 s}) and per-row scale freedom,
    the unnormalized weight is w[i,j] = max(R1_i * E2_j, E2a_j) where
    R1 = e^{0.8 Wh1}, E2 = e^{Wh2}, E2a = e^{0.2 Wh2}.  Branch 1 wins iff
    Wh1_i + Wh2_j >= 0 — a THRESHOLD in Wh2_j.  Sorting j by Wh2 descending
    makes every row's weight vector "R1_i * E2_j for the first k_i sorted j,
    then E2a_j", so with prefix tables over sorted j
        P1[k,:] = sum_{r<k} E2_r * [Wh_r | 1]
        P2[k,:] = sum_{r<k} E2a_r * [Wh_r | 1]
    the full softmax-matmul collapses to (exactly)
        out[i,:] = elu( (R1_i*P1[k_i,:64] + S2[:64]-P2[k_i,:64])
                        / (R1_i*P1[k_i,64] + S2[64]-P2[k_i,64]) ).
    At the branch boundary both branches are equal, so fp-rounding-induced
    misassignments of k_i are harmless (continuous).

Host does O(N log N) metadata only: two matvecs h@(W a1), h@(W a2) (0.1% of
the layer FLOPs) to derive the sort permutation, thresholds k_i and the
core assignment; all O(N*D) table math runs on device.

Sharding: j-blocks of 1024 sorted rows per core (each core builds its own
1024-row slice of the tables from its own 0.5 MB hT slice — no full-hT
replication).  Output rows are assigned to the core OWNING their k_i block,
so the table gather is core-local (HBM round trip, no cross-core traffic).
Only the 8x130-float block totals cross cores (one tiny AllGather), hidden
behind the table write + gather.

Device pipeline per core: project own j-block -> scale by E2/E2a (DVE) ->
in-tile inclusive prefix via one [128x128] upper-triangular matmul per tile
-> tables to HBM -> dma_gather of the k_i rows -> epilogue combines with
tile/block offsets (folded in via a host-provided one-hot matmul) -> ELU.
"""

import functools
import os

import numpy as np

os.environ.setdefault("NEURON_RT_RESET_CORES", "1")

N = 8192
IN_DIM = 256
OUT_DIM = 64
ALPHA = 0.2
NCORES = 8
P = 128
KC = IN_DIM // P            # 2 contraction chunks
BLK = N // NCORES           # 1024 sorted j rows per core
JT = BLK // P               # 8 j-tiles per core
DA = OUT_DIM + 1            # 65 = [Wh | ones]
TW2 = 2 * DA                # 130 = [P1 | P2] row
TW = 192                    # table row pitch (fp32); 768 B, 256-aligned
TBH = BLK + 8               # 1032 table rows (row 0 = zeros, 1..1024 prefixes)
CAP = 1536                  # max query rows per core (measured max 1135)
QT = CAP // P               # 12 query tiles
GHALF = CAP // 2            # hw dma_gather handles at most 1024 indices


def build_nc(repeat: int = 1):
    import concourse.mybir as mybir
    import concourse.tile as tile
    from concourse import bacc
    from concourse.masks import make_identity, make_upper_triangular

    fp32 = mybir.dt.float32
    bf16 = mybir.dt.bfloat16
    i16 = mybir.dt.int16
    Alu = mybir.AluOpType
    Act = mybir.ActivationFunctionType

    nc = bacc.Bacc("TRN2", target_bir_lowering=False, debug=False,
                   num_devices=NCORES)

    hTs_d = nc.dram_tensor("hTs", [IN_DIM, BLK], bf16, kind="ExternalInput")
    hTq_d = nc.dram_tensor("hTq", [IN_DIM, CAP], bf16, kind="ExternalInput")
    waug_d = nc.dram_tensor("waug", [IN_DIM, 68], bf16, kind="ExternalInput")
    idx_d = nc.dram_tensor("idx", [P, CAP // 16], i16, kind="ExternalInput")
    q1_d = nc.dram_tensor("q1hot", [JT, CAP], fp32, kind="ExternalInput")
    sel_d = nc.dram_tensor("sel", [NCORES, 2], fp32, kind="ExternalInput")
    out_d = nc.dram_tensor("out", [CAP, OUT_DIM], fp32, kind="ExternalOutput")

    hTs_r = hTs_d.ap().rearrange("(c p) j -> p c j", p=P)
    hTq_r = hTq_d.ap().rearrange("(c p) j -> p c j", p=P)
    waug_r = waug_d.ap().rearrange("(c p) d -> p c d", p=P)
    out_r = out_d.ap().rearrange("(q p) d -> p q d", p=P)

    with tile.TileContext(nc) as tc:
        with (
            tc.tile_pool(name="singles", bufs=1) as singles,
            tc.tile_pool(name="inp", bufs=1) as inp,
            tc.tile_pool(name="work", bufs=2) as work,
            tc.tile_pool(name="ep", bufs=2) as ep,
            tc.tile_pool(name="ps_a", bufs=2, space="PSUM") as ps_a,
            tc.tile_pool(name="ps_pref", bufs=2, space="PSUM") as ps_pref,
            tc.tile_pool(name="ps_small", bufs=2, space="PSUM") as ps_small,
            tc.tile_pool(name="ps_qoff", bufs=2, space="PSUM") as ps_qoff,
            tc.tile_pool(name="dram_tbl", bufs=2, space="DRAM") as dram_tbl,
            tc.tile_pool(name="dram", bufs=2, space="DRAM") as dram,
        ):
            # ---- constants (built once) ----------------------------------
            ident = singles.tile([P, P], fp32)
            make_identity(nc, ident)
            u128 = singles.tile([P, P], fp32)           # U[j,k]=1 iff j<=k
            make_upper_triangular(nc, u128, 1.0, diag=True)
            u9 = singles.tile([JT, JT + 1], fp32)       # U9[r,t]=1 iff r<t
            nc.gpsimd.memset(u9[:], 0.0)
            nc.gpsimd.affine_select(
                out=u9[:], in_=u9[:], compare_op=Alu.is_ge,
                fill=1.0, base=0, pattern=[[-1, JT + 1]], channel_multiplier=1)
            ones1_8 = singles.tile([1, NCORES], fp32)
            nc.vector.memset(ones1_8[:], 1.0)
            i8neg = singles.tile([NCORES, NCORES], fp32)
            nc.vector.tensor_scalar(i8neg[:], ident[0:NCORES, 0:NCORES],
                                    -1.0, None, Alu.mult)
            zrow = singles.tile([1, TW], fp32)
            nc.vector.memset(zrow[:], 0.0)

            for _rep in range(repeat):
                # ---- load inputs ------------------------------------------
                waug_sb = inp.tile([P, KC, 68], bf16, tag="waug")
                nc.sync.dma_start(waug_sb[:], waug_r)
                hTs_sb = inp.tile([P, KC, BLK], bf16, tag="hTs")
                nc.sync.dma_start(hTs_sb[:], hTs_r)
                hTq_sb = inp.tile([P, KC, CAP], bf16, tag="hTq")
                nc.sync.dma_start(hTq_sb[:], hTq_r)
                idx_sb = inp.tile([P, CAP // 16], i16, tag="idx")
                nc.sync.dma_start(idx_sb[:], idx_d.ap())
                q1_sb = inp.tile([JT, CAP], fp32, tag="q1")
                nc.sync.dma_start(q1_sb[:], q1_d.ap())
                sel_sb = inp.tile([NCORES, 2], fp32, tag="sel")
                nc.sync.dma_start(sel_sb[:], sel_d.ap())

                # ---- project own sorted j-block; exp scales ---------------
                whs = work.tile([P, JT, DA], fp32, tag="whs")
                nc.vector.memset(whs[:, :, OUT_DIM], 1.0)
                e2 = work.tile([P, JT], fp32, tag="e2")
                e2a = work.tile([P, JT], fp32, tag="e2a")
                x = work.tile([P, JT, TW2], fp32, tag="x")
                for t in range(JT):
                    ps = ps_a.tile([P, 68], fp32, tag="proj")
                    for c in range(KC):
                        nc.tensor.matmul(
                            ps[:, 0:66],
                            hTs_sb[:, c, t * P:(t + 1) * P],
                            waug_sb[:, c, 0:66],
                            start=(c == 0), stop=(c == KC - 1))
                    nc.scalar.activation(whs[:, t, 0:OUT_DIM],
                                         ps[:, 0:OUT_DIM], Act.Copy)
                    nc.scalar.activation(e2[:, t:t + 1], ps[:, 65:66], Act.Exp)
                    nc.scalar.activation(e2a[:, t:t + 1], ps[:, 65:66],
                                         Act.Exp, scale=ALPHA)
                    # x = [E2*Whaug | E2a*Whaug]
                    nc.vector.tensor_scalar(x[:, t, 0:DA], whs[:, t, :],
                                            e2[:, t:t + 1], None, Alu.mult)
                    nc.vector.tensor_scalar(x[:, t, DA:TW2], whs[:, t, :],
                                            e2a[:, t:t + 1], None, Alu.mult)

                # ---- in-tile inclusive prefix (one tri-matmul per tile) ---
                tbl_sb = work.tile([P, JT, TW], fp32, tag="tbl")
                nc.vector.memset(tbl_sb[:, :, TW2:TW], 0.0)
                for t in range(JT):
                    psp = ps_pref.tile([P, TW2], fp32, tag="pref")
                    nc.tensor.matmul(psp[:], u128[:], x[:, t, :],
                                     start=True, stop=True)
                    nc.scalar.activation(tbl_sb[:, t, 0:TW2], psp[:], Act.Copy)

                # ---- tile totals -> partitions; local offsets + core total
                totrows = work.tile([JT, TW2], fp32, tag="totrows")
                for t in range(JT):
                    nc.sync.dma_start(totrows[t:t + 1, :],
                                      tbl_sb[P - 1:P, t, 0:TW2])
                # one shared PSUM bank for the small intermediates:
                # off [0:9, 0:130] | selA [0:1,130:260] | selB [0:1,260:390]
                # | whq [0:128, 390:402]
                psm = ps_small.tile([P, 512], fp32, tag="small")
                pso = psm[0:JT + 1, 0:TW2]
                nc.tensor.matmul(pso, u9[:], totrows[:], start=True,
                                 stop=True)
                off_sb = work.tile([JT + 1, TW2], fp32, tag="off_sb")
                nc.scalar.activation(off_sb[:], pso, Act.Copy)

                # ---- tiny collective: allgather core totals ---------------
                variant = os.environ.get("BASS_GAT_VARIANT", "full")
                tots_all = work.tile([NCORES, TW2], fp32, tag="tots_all")
                if "nocc" in variant:
                    nc.vector.memset(tots_all[:], 0.0)
                else:
                    cc_in = dram.tile([1, TW2], fp32, tag="cc_in")
                    nc.sync.dma_start(cc_in[:], off_sb[JT:JT + 1, :])
                    cc_out = dram.tile([NCORES, TW2], fp32, tag="cc_out")
                    nc.gpsimd.collective_compute(
                        "AllGather", mybir.AluOpType.bypass,
                        replica_groups=[list(range(NCORES))],
                        ins=[cc_in.opt()], outs=[cc_out.opt()])
                    nc.sync.dma_start(tots_all[:], cc_out[:])

                # blockoff row (sum over c'<c) and suffix row (sum c'>=c)
                pssA = psm[0:1, 130:260]
                nc.tensor.matmul(pssA, sel_sb[:, 0:1], tots_all[:],
                                 start=True, stop=True)
                sgA = work.tile([1, TW2], fp32, tag="sgA")
                nc.scalar.activation(sgA[:], pssA, Act.Copy)
                pssB = psm[0:1, 260:390]
                nc.tensor.matmul(pssB, sel_sb[:, 1:2], tots_all[:],
                                 start=True, stop=True)
                sgB = work.tile([1, TW2], fp32, tag="sgB")
                nc.scalar.activation(sgB[:], pssB, Act.Copy)

                # offabs[t, 0:65]  = blockoff1 + localoff1[t]
                # offabs[t, 65:130]= (Sg2 - blockoff2) - localoff2[t]
                psoa_t = ps_pref.tile([P, TW2], fp32, tag="pref")
                psoa = psoa_t[0:NCORES, :]
                nc.tensor.matmul(psoa[:, 0:DA], ones1_8[:], sgA[0:1, 0:DA],
                                 start=True, stop=False)
                nc.tensor.matmul(psoa[:, 0:DA], ident[0:NCORES, 0:NCORES],
                                 off_sb[0:JT, 0:DA], start=False, stop=True)
                nc.tensor.matmul(psoa[:, DA:TW2], ones1_8[:],
                                 sgB[0:1, DA:TW2], start=True, stop=False)
                nc.tensor.matmul(psoa[:, DA:TW2], i8neg[:],
                                 off_sb[0:JT, DA:TW2], start=False, stop=True)
                offabs_sb = work.tile([NCORES, TW2], fp32, tag="offabs_sb")
                nc.scalar.activation(offabs_sb[:], psoa[:], Act.Copy)

                # ---- query-row R1 -----------------------------------------
                psq = psm[:, 390:390 + QT]
                for t in range(QT):
                    for c in range(KC):
                        nc.tensor.matmul(
                            psq[:, t:t + 1],
                            hTq_sb[:, c, t * P:(t + 1) * P],
                            waug_sb[:, c, 64:65],
                            start=(c == 0), stop=(c == KC - 1))
                r1q = work.tile([P, QT], fp32, tag="r1q")
                nc.scalar.activation(r1q[:], psq[:], Act.Exp, scale=0.8)

                # ---- write table to HBM; gather the k_i rows --------------
                tbl_dram = dram_tbl.tile([TBH, TW], fp32, tag="tbl_dram")
                tbl_dst = tbl_dram[1:BLK + 1, :].rearrange(
                    "(t p) w -> p t w", p=P)
                nc.sync.dma_start(tbl_dst, tbl_sb[:])
                nc.sync.dma_start(tbl_dram[0:1, :], zrow[:])
                gath = ep.tile([P, QT, TW], fp32, tag="gath")
                if "nogather" in variant:
                    nc.vector.memset(gath[:], 1.0)
                else:
                    # hw dma_gather caps num_idxs at 1024 -> split in halves
                    for g in range(2):
                        nc.gpsimd.dma_gather(
                            out_ap=gath[:, g * (GHALF // P):(g + 1) * (GHALF // P), :],
                            in_ap=tbl_dram[:],
                            idxs_ap=idx_sb[:, g * (GHALF // 16):(g + 1) * (GHALF // 16)],
                            num_idxs=GHALF,
                            num_idxs_reg=GHALF,
                            elem_size=TW)

                # ---- per-query offsets via one-hot matmul -----------------
                qoff_sb = ep.tile([P, QT, TW2], fp32, tag="qoff")
                for q in range(QT):
                    psqo = ps_qoff.tile([P, TW2], fp32, tag="qoff")
                    nc.tensor.matmul(psqo[:], q1_sb[:, q * P:(q + 1) * P],
                                     offabs_sb[:], start=True, stop=True)
                    nc.scalar.activation(qoff_sb[:, q, :], psqo[:], Act.Copy)

                # ---- epilogue: comb = r1q*(G+qoff1) + (qoff2 - H) ---------
                t0 = ep.tile([P, QT, DA], fp32, tag="t0")
                nc.vector.tensor_tensor(t0[:], gath[:, :, 0:DA],
                                        qoff_sb[:, :, 0:DA], Alu.add)
                t2 = ep.tile([P, QT, DA], fp32, tag="t2")
                nc.vector.tensor_tensor(t2[:], qoff_sb[:, :, DA:TW2],
                                        gath[:, :, DA:TW2], Alu.subtract)
                comb = ep.tile([P, QT, DA], fp32, tag="comb")
                for q in range(QT):
                    nc.vector.scalar_tensor_tensor(
                        comb[:, q, :], t0[:, q, :], r1q[:, q:q + 1],
                        t2[:, q, :], Alu.mult, Alu.add)

                zinv = ep.tile([P, QT], fp32, tag="zinv")
                nc.vector.reciprocal(zinv[:], comb[:, :, OUT_DIM])
                outv = ep.tile([P, QT, OUT_DIM], fp32, tag="outv")
                for q in range(QT):
                    nc.vector.tensor_scalar(
                        outv[:, q, :], comb[:, q, 0:OUT_DIM],
                        zinv[:, q:q + 1], None, Alu.mult)

                # ELU, exact: (max(x,0) - 1) + exp(min(x,0))
                flat = outv.rearrange("p q d -> p (q d)")
                r = ep.tile([P, QT * OUT_DIM], fp32, tag="elur")
                m = ep.tile([P, QT * OUT_DIM], fp32, tag="elum")
                nc.vector.tensor_scalar(r[:], flat, 0.0, -1.0, Alu.max,
                                        Alu.add)
                nc.vector.tensor_scalar(m[:], flat, 0.0, None, Alu.min)
                nc.scalar.activation(m[:], m[:], Act.Exp)
                nc.vector.tensor_tensor(flat, r[:], m[:], Alu.add)

                nc.sync.dma_start(out_r, outv[:])

    nc.compile()
    return nc


@functools.lru_cache(maxsize=4)
def _cached_nc(repeat: int = 1):
    return build_nc(repeat)


class _Runner:
    """Compile once, load once, execute many times on the 8 cores."""

    def __init__(self, repeat: int = 1):
        import jax
        from jax.experimental.shard_map import shard_map
        from jax.sharding import Mesh, NamedSharding, PartitionSpec
        import concourse.mybir as mybir
        from concourse import bass2jax

        self.jax = jax
        nc = _cached_nc(repeat)
        partition_name = (nc.partition_id_tensor.name
                          if nc.partition_id_tensor else None)
        bass2jax.install_neuronx_cc_hook()

        in_names, out_names, out_avals, zero_outs = [], [], [], []
        for alloc in nc.m.functions[0].allocations:
            if not isinstance(alloc, mybir.MemoryLocationSet):
                continue
            name = alloc.memorylocations[0].name
            if alloc.kind == "ExternalInput":
                if name != partition_name:
                    in_names.append(name)
            elif alloc.kind == "ExternalOutput":
                shape = tuple(alloc.tensor_shape)
                dt = mybir.dt.np(alloc.dtype)
                out_names.append(name)
                out_avals.append(jax.core.ShapedArray(shape, dt))
                zero_outs.append(np.zeros((NCORES * shape[0], *shape[1:]), dt))
        self.in_names = in_names
        self.out_names = out_names
        self.out_shapes = [tuple(a.shape) for a in out_avals]
        all_names = tuple(in_names + out_names)
        if partition_name is not None:
            all_names = all_names + (partition_name,)

        def _body(*args):
            operands = list(args)
            if partition_name is not None:
                operands.append(bass2jax.partition_id_tensor())
            outs = bass2jax._bass_exec_p.bind(
                *operands,
                out_avals=tuple(out_avals),
                in_names=all_names,
                out_names=tuple(out_names),
                lowering_input_output_aliases=(),
                sim_require_finite=True,
                sim_require_nnan=True,
                nc=nc,
            )
            return tuple(outs)

        devices = jax.devices()[:NCORES]
        mesh = Mesh(np.asarray(devices), ("core",))
        n_args = len(in_names) + len(out_names)
        self.fn = jax.jit(
            shard_map(
                _body, mesh=mesh,
                in_specs=(PartitionSpec("core"),) * n_args,
                out_specs=(PartitionSpec("core"),) * len(out_names),
                check_rep=False,
            ),
            keep_unused=True,
        )
        self.sharding = NamedSharding(mesh, PartitionSpec("core"))
        self.zero_dev = [jax.device_put(z, self.sharding) for z in zero_outs]
        self.dev_inputs = None
        self._inputs_key = None

    def set_inputs(self, in_maps):
        key = id(in_maps)
        if self._inputs_key == key and self.dev_inputs is not None:
            return
        concat = [
            np.concatenate([np.asarray(m[name]) for m in in_maps], axis=0)
            for name in self.in_names
        ]
        self.dev_inputs = [
            self.jax.device_put(c, self.sharding) for c in concat
        ]
        self.jax.block_until_ready(self.dev_inputs)
        self._inputs_key = key

    def execute(self):
        outs = self.fn(*self.dev_inputs, *self.zero_dev)
        self.jax.block_until_ready(outs)
        return outs

    def results(self):
        outs = self.execute()
        per_core = []
        for c in range(NCORES):
            per_core.append({
                name: np.asarray(outs[i]).reshape(
                    NCORES, *self.out_shapes[i])[c]
                for i, name in enumerate(self.out_names)
            })
        return per_core


@functools.lru_cache(maxsize=4)
def _cached_runner(repeat: int = 1):
    return _Runner(repeat)


def _bf16_np():
    import concourse.mybir as mybir
    return mybir.dt.np(mybir.dt.bfloat16)


def _marshal(h, W, a):
    """Host metadata + sharded layout.  Returns (in_maps, unperm) where
    unperm[c] = original row indices of core c's query slots."""
    bf = _bf16_np()
    h = np.asarray(h, dtype=np.float32)
    W = np.asarray(W, dtype=np.float32)
    a = np.asarray(a, dtype=np.float32).reshape(2 * OUT_DIM, 1)
    wa1 = (W @ a[:OUT_DIM]).astype(np.float32)          # [256,1]
    wa2 = (W @ a[OUT_DIM:]).astype(np.float32)
    waug = np.ascontiguousarray(
        np.concatenate([W, wa1, wa2, np.zeros((IN_DIM, 2), np.float32)],
                       axis=1)).astype(bf)              # [256, 68]

    wh1 = (h @ wa1)[:, 0]                               # [N]
    wh2 = (h @ wa2)[:, 0]
    perm = np.argsort(-wh2, kind="stable")
    wh2s = wh2[perm]
    k = np.searchsorted(-wh2s, wh1, side="right")       # [N] in [0, N]
    core = np.clip(k // BLK, 0, NCORES - 1).astype(np.int64)
    loc = (k - core * BLK).astype(np.int64)             # [0, 1024]

    hT = np.ascontiguousarray(h.T).astype(bf)           # [256, 8192]
    hTsorted = hT[:, perm]

    in_maps = []
    unperm = []
    for c in range(NCORES):
        rows = np.nonzero(core == c)[0]
        n_c = len(rows)
        assert n_c <= CAP, (c, n_c)
        unperm.append(rows)
        hTq = np.zeros((IN_DIM, CAP), bf)
        hTq[:, :n_c] = hT[:, rows]
        lvals = np.zeros(CAP, np.int16)
        lvals[:n_c] = loc[rows].astype(np.int16)
        # slot n reads its index at partition n%16 + base, col n//16; the
        # base is 0 in CoreSim but 16 on hardware's channel reader, so
        # replicate the 16-row wrap across all partition groups.  Two
        # independent halves (hw gather caps at 1024 indices).
        idx = np.zeros((P, CAP // 16), np.int16)
        for g in range(2):
            w = lvals[g * GHALF:(g + 1) * GHALF].reshape(GHALF // 16, 16).T
            idx[:, g * (GHALF // 16):(g + 1) * (GHALF // 16)] = np.tile(
                w, (P // 16, 1))
        q1 = np.zeros((JT, CAP), np.float32)
        tq = np.zeros(CAP, np.int64)                    # tile of each query
        tq[:n_c] = np.maximum(loc[rows] - 1, 0) // P    # (pads -> tile 0,
        q1[tq, np.arange(CAP)] = 1.0                    #  keeps den > 0)
        sel = np.zeros((NCORES, 2), np.float32)
        sel[:c, 0] = 1.0
        sel[c:, 1] = 1.0
        in_maps.append({
            "hTs": np.ascontiguousarray(hTsorted[:, c * BLK:(c + 1) * BLK]),
            "hTq": hTq,
            "waug": waug,
            "idx": idx,
            "q1hot": q1,
            "sel": sel,
        })
    return in_maps, unperm


def run_on_cores(in_maps, repeat: int = 1):
    runner = _cached_runner(repeat)
    runner.set_inputs(in_maps)
    return runner.results()


def _kernel_numpy_exact(h, W, a):
    """Defensive fallback for out-of-distribution inputs (bucket overflow).
    Exact reference math on host; never taken for spec-distribution inputs."""
    h = np.asarray(h, np.float64)
    W = np.asarray(W, np.float64)
    a = np.asarray(a, np.float64).reshape(2 * OUT_DIM, 1)
    Wh = h @ W
    wh1 = Wh @ a[:OUT_DIM]
    wh2 = Wh @ a[OUT_DIM:]
    out = np.empty((N, OUT_DIM), np.float64)
    for s in range(0, N, 512):
        e = wh1[s:s + 512] + wh2.T
        e = np.where(e > 0, e, ALPHA * e)
        e -= e.max(axis=1, keepdims=True)
        w = np.exp(e)
        out[s:s + 512] = (w @ Wh) / w.sum(axis=1, keepdims=True)
    return np.where(out > 0, out, np.expm1(out)).astype(np.float32)


def kernel(h, adj, W, a):
    import time
    try:
        in_maps, unperm = _marshal(h, W, a)
    except AssertionError:
        return _kernel_numpy_exact(h, W, a)
    res = None
    last_exc = None
    for attempt in range(4):
        try:
            res = run_on_cores(in_maps, repeat=1)
            break
        except Exception as e:
            last_exc = e
            _cached_runner.cache_clear()
            _cached_nc.cache_clear()
            time.sleep(20 * (attempt + 1))
    if res is None:
        raise last_exc
    out = np.empty((N, OUT_DIM), np.float32)
    for c in range(NCORES):
        rows = unperm[c]
        out[rows] = res[c]["out"][:len(rows)]
    return out


if __name__ == "__main__":
    rng = np.random.default_rng(0)
    h = rng.standard_normal((N, IN_DIM), dtype=np.float32)
    W = (rng.standard_normal((IN_DIM, OUT_DIM), dtype=np.float32) * 0.1)
    a = (rng.standard_normal((2 * OUT_DIM, 1), dtype=np.float32) * 0.1)
    adj = np.zeros((N, N), dtype=bool)
    out = kernel(h, adj, W, a)
    print("out", out.shape, out.dtype, float(out.mean()))
